# revision 6
# baseline (speedup 1.0000x reference)
"""Multi-head attention TRN2 Bass kernel (8 NeuronCores, tensor-parallel).

Sharding: Megatron-style TP over (batch x head-group). 8 cores = 2 batches x 4
head-groups of 4 heads each. Each core computes its heads' Q/K/V projections,
masked-softmax attention, and a partial output projection; the host sums the 4
partials per batch (the TP unshard).

v2 design (single fused pipeline, fp16 compute):
  - One global step machine over (mh, h, nt): scores (PE, K=64 via partition-
    offset tiles, no zero-padding), exp (ACT, scale=1/8 folded, PSUM->SBUF
    fp16), mask multiply (DVE, nt-PAIRED 2048-wide), ctx (PE, lhsT=[vw_h|1]
    65 rows, lagging LAG steps behind scores).
  - v-projection, q-mh1 projection and out-proj(mh0) are injected as side
    quanta between attention steps so PE never idles between phases.
  - k + q(mh0) projections stream during lead-in with k/q DMA interleaved.
  - Normalize per head: pctx -> csb (DVE+ACT halves, parallel drain), ones-
    broadcast MM into utility PSUM, reciprocal_approx_fast, one multiply.
  - PSUM: scores 2x[128,1024] (4 banks) + pctx [65,1024] (2) + util 2x[128,512]
    (2) = 8 banks; the lead-in projection pool closes before attention opens.
"""
import os
import sys

for p in ("/opt/trn_rl_repo",):
    if p not in sys.path:
        sys.path.insert(0, p)

from contextlib import ExitStack

import numpy as np

import concourse.bass as bass
import concourse.tile as tile
from concourse import bacc, mybir
from concourse.bass_utils import run_bass_kernel_spmd

F32 = mybir.dt.float32
F16 = mybir.dt.float16
EXP = mybir.ActivationFunctionType.Exp
ts = bass.ts

B, M, N, E = 2, 2048, 2048, 1024  # batch, q-len, k-len, d_model
H, DK = 16, 64                    # heads, head dim
NCORES = 8
GROUPS = 4                        # head groups (cores per batch)
DLOC = (H // GROUPS) * DK         # 256 per-core projection width
HL = H // GROUPS                  # 4 local heads
ET = E // 128                     # 8 k-tiles of the projection contraction
NT = N // 128                     # 16 n-tiles
VSTR = HL * (DK + 1)              # 260: vw slot stride per n-tile
MH = 2                            # m halves of 1024

LAG = int(os.environ.get("K_LAG", "4"))          # ctx lag behind scores


def build_program() -> bass.Bass:
    nc = bacc.Bacc()

    qT_d = nc.dram_tensor("qT", [E, M], F16, kind="ExternalInput")
    kT_d = nc.dram_tensor("kT", [E, N], F16, kind="ExternalInput")
    vT_d = nc.dram_tensor("vT", [E, N], F16, kind="ExternalInput")
    keepT_d = nc.dram_tensor("keepT", [N, M], F16, kind="ExternalInput")
    wqT_d = nc.dram_tensor("wqT", [E, DLOC], F16, kind="ExternalInput")
    wkT_d = nc.dram_tensor("wkT", [E, DLOC], F16, kind="ExternalInput")
    wvT_d = nc.dram_tensor("wvT", [E, DLOC], F16, kind="ExternalInput")
    woT_d = nc.dram_tensor("woT", [DLOC, E], F16, kind="ExternalInput")
    out_d = nc.dram_tensor("out", [M, E], F16, kind="ExternalOutput")

    with tile.TileContext(nc) as tc, ExitStack() as ctx:
        const_pool = ctx.enter_context(tc.tile_pool(name="const", bufs=1))
        w_pool = ctx.enter_context(tc.tile_pool(name="weights", bufs=1))
        act_pool = ctx.enter_context(tc.tile_pool(name="acts", bufs=1))

        ones64 = const_pool.tile([1, 64], F16)
        nc.vector.memset(ones64[:], 1.0)
        warm_exp = const_pool.tile([1, 64], F16)
        nc.scalar.activation(warm_exp[:], ones64[:], EXP, scale=0.125)

        wq_sb = w_pool.tile([128, ET * DLOC], F16, tag="wq")
        wk_sb = w_pool.tile([128, ET * DLOC], F16, tag="wk")
        wv_sb = w_pool.tile([128, ET * DLOC], F16, tag="wv")
        wo_sb = w_pool.tile([128, 2 * E], F16, tag="wo")

        # qw/kw: [d, m] / [d, n]; d2 indexes the two 128-row halves of DLOC;
        # within a half, rows 0-63 are the even head, 64-127 the odd head.
        qw = [act_pool.tile([128, M], F16, tag=f"qw{i}", name=f"qw{i}") for i in range(2)]
        kw = [act_pool.tile([128, N], F16, tag=f"kw{i}", name=f"kw{i}") for i in range(2)]
        vw_sb = act_pool.tile([128, NT * VSTR], F16, tag="vw")
        ctxs = [act_pool.tile([128, M], F16, tag=f"ctx{i}", name=f"ctx{i}") for i in range(2)]
        nc.vector.memset(vw_sb[:], 1.0)  # ones cols persist; data overwritten
        vw3 = vw_sb.rearrange("p (s x) -> p s x", x=DK + 1)

        # long-lived pools
        util_ps = ctx.enter_context(tc.tile_pool(name="util_ps", bufs=2, space="PSUM"))
        xq_pool = ctx.enter_context(tc.tile_pool(name="xq", bufs=16))
        vT_pool = ctx.enter_context(tc.tile_pool(name="vTp", bufs=8))
        keep_pool = ctx.enter_context(tc.tile_pool(name="keep", bufs=3))
        au_pool = ctx.enter_context(tc.tile_pool(name="au", bufs=2))
        am_pool = ctx.enter_context(tc.tile_pool(name="am", bufs=3))
        csb_pool = ctx.enter_context(tc.tile_pool(name="csb", bufs=2))
        rbs_pool = ctx.enter_context(tc.tile_pool(name="rbs", bufs=1))
        osb_pool = ctx.enter_context(tc.tile_pool(name="osb", bufs=3))

        # ---- lead-in: k (both halves) + q (mh0) projections ----
        with tc.tile_pool(name="lead_ps", bufs=4, space="PSUM") as lead_ps:
            # weights first, then interleaved k-h0/q-mh0 tiles so both stream
            for et in range(ET):
                nc.sync.dma_start(wk_sb[:, ts(et, DLOC)], wkT_d[ts(et, 128), :])
            for et in range(ET):
                nc.sync.dma_start(wq_sb[:, ts(et, DLOC)], wqT_d[ts(et, 128), :])
            k0ts, q0ts = [], []
            for et in range(ET):
                kt0 = xq_pool.tile([128, 1024], F16, tag="xq", name=f"kt0_{et}")
                nc.sync.dma_start(kt0[:], kT_d[ts(et, 128), 0:1024])
                k0ts.append(kt0)
                qt0 = xq_pool.tile([128, 1024], F16, tag="xq", name=f"qt0_{et}")
                nc.sync.dma_start(qt0[:], qT_d[ts(et, 128), 0:1024])
                q0ts.append(qt0)
            # v weights + vT stream next (needed by injected v-proj quanta)
            for et in range(ET):
                nc.sync.dma_start(wv_sb[:, ts(et, DLOC)], wvT_d[ts(et, 128), :])
            vts = []
            for et in range(ET):
                vt = vT_pool.tile([128, N], F16, tag="vT", name=f"vt{et}")
                nc.sync.dma_start(vt[:], vT_d[ts(et, 128), :])
                vts.append(vt)
            # k half1
            k1ts = []
            for et in range(ET):
                kt1 = xq_pool.tile([128, 1024], F16, tag="xq", name=f"kt1_{et}")
                nc.sync.dma_start(kt1[:], kT_d[ts(et, 128), 1024:2048])
                k1ts.append(kt1)

            def proj_qk_unit(w_sb, dst, chalf, xts):
                """One 1024-col half of a q/k projection: 32 MMs + 4 copies."""
                pss = []
                for j in range(4):
                    ps = lead_ps.tile([128, 512], F32, tag="lp", name=f"lp{j}")
                    pss.append(ps)
                for et in range(ET):
                    for d2 in range(2):
                        for c2 in range(2):
                            nc.tensor.matmul(
                                pss[d2 * 2 + c2][:],
                                w_sb[:, et * DLOC + d2 * 128 : et * DLOC + (d2 + 1) * 128],
                                xts[et][:, ts(c2, 512)],
                                start=(et == 0), stop=(et == ET - 1),
                            )
                for j, ps in enumerate(pss):
                    d2, c2 = divmod(j, 2)
                    nc.vector.tensor_copy(
                        dst[d2][:, chalf * 1024 + c2 * 512 : chalf * 1024 + (c2 + 1) * 512],
                        ps[:],
                    )

            proj_qk_unit(wk_sb, kw, 0, k0ts)
            proj_qk_unit(wq_sb, qw, 0, q0ts)
            proj_qk_unit(wk_sb, kw, 1, k1ts)

        # keep mask DMA: tiles per (mh, nthalf) of [128, 8*1024]
        keep_tiles = {}

        def dma_keep(mh, nh):
            kt = keep_pool.tile([128, 8 * 1024], F16, tag="keep", name=f"keep{mh}{nh}")
            for j in range(8):
                nt = nh * 8 + j
                nc.sync.dma_start(
                    kt[:, ts(j, 1024)],
                    keepT_d[ts(nt, 128), mh * 1024 : (mh + 1) * 1024],
                )
            keep_tiles[(mh, nh)] = kt

        dma_keep(0, 0)
        dma_keep(0, 1)

        # ---- side-work quanta (emitted between attention steps) ----
        def v_quantum(g):
            def emit():
                pss = []
                for j in range(2):
                    ps = util_ps.tile([128, 512], F32, tag="u", name=f"vp{j}")
                    pss.append(ps)
                for et in range(ET):
                    for j in range(2):
                        nt = g * 2 + j
                        nc.tensor.matmul(
                            pss[j][:, 0:DLOC],
                            vts[et][:, ts(nt, 128)],
                            wv_sb[:, ts(et, DLOC)],
                            start=(et == 0), stop=(et == ET - 1),
                        )
                for j in range(2):
                    nt = g * 2 + j
                    nc.vector.tensor_copy(
                        vw3[:, nt * HL : (nt + 1) * HL, 0:DK],
                        pss[j][:, 0:DLOC].rearrange("p (s x) -> p s x", x=DK),
                    )
            return emit

        q1_xts = []

        def q1_quantum(d2):
            def emit():
                if d2 == 0:
                    for et in range(ET):
                        qt1 = xq_pool.tile([128, 1024], F16, tag="xq", name=f"qt1_{et}")
                        nc.sync.dma_start(qt1[:], qT_d[ts(et, 128), 1024:2048])
                        q1_xts.append(qt1)
                pss = []
                for c2 in range(2):
                    ps = util_ps.tile([128, 512], F32, tag="u", name=f"qp{c2}")
                    pss.append(ps)
                for et in range(ET):
                    for c2 in range(2):
                        nc.tensor.matmul(
                            pss[c2][:],
                            wq_sb[:, et * DLOC + d2 * 128 : et * DLOC + (d2 + 1) * 128],
                            q1_xts[et][:, ts(c2, 512)],
                            start=(et == 0), stop=(et == ET - 1),
                        )
                for c2 in range(2):
                    nc.vector.tensor_copy(
                        qw[d2][:, 1024 + c2 * 512 : 1024 + (c2 + 1) * 512], pss[c2][:]
                    )
            return emit

        def oproj_quantum(mh, q):
            def emit():
                if (mh, q) == (0, 0):
                    for kt2 in range(2):
                        nc.sync.dma_start(
                            wo_sb[:, ts(kt2, E)], woT_d[ts(kt2, 128), :]
                        )
                for i in range(2):
                    mt = mh * 8 + q * 2 + i
                    for ec in range(2):
                        po = util_ps.tile([128, 512], F32, tag="u", name="po")
                        for kt2 in range(2):
                            nc.tensor.matmul(
                                po[:],
                                ctxs[kt2][:, ts(mt, 128)],
                                wo_sb[:, kt2 * E + ec * 512 : kt2 * E + (ec + 1) * 512],
                                start=(kt2 == 0), stop=(kt2 == 1),
                            )
                        ob = osb_pool.tile([128, 512], F16, tag="ob", name="ob")
                        if (i * 2 + ec) % 2 == 0:
                            nc.vector.tensor_copy(ob[:], po[:])
                        else:
                            nc.scalar.copy(ob[:], po[:])
                        nc.sync.dma_start(out_d[ts(mt, 128), ts(ec, 512)], ob[:])
            return emit

        # schedule: global step s in [0, 128): (mh, h, nt); injections run
        # before the step's scores so their PE work lands between steps.
        injections = {}
        for g in range(8):
            injections.setdefault(2 * g, []).append(v_quantum(g))
        injections.setdefault(18, []).append(q1_quantum(0))
        injections.setdefault(22, []).append(q1_quantum(1))
        # out-proj mh0 during mh1 steps; mh0 h3's last ctx+normalize is
        # emitted at step 63+LAG, so start strictly after that.
        for q in range(4):
            injections.setdefault(68 + LAG + 4 * q, []).append(oproj_quantum(0, q))
        prefetches = {40: (1, 0), 52: (1, 1)}

        s_ps = ctx.enter_context(tc.tile_pool(name="s_ps", bufs=2, space="PSUM"))
        c_ps = ctx.enter_context(tc.tile_pool(name="c_ps", bufs=1, space="PSUM"))

        am_half = {}   # step -> (am_pair, half_idx)
        au_cur = [None]
        pctx_cur = [None]

        def head_of(s):
            mh, r = divmod(s, 64)
            return mh, r // 16, r % 16

        def emit_scores_exp_mask(s):
            mh, h, nt = head_of(s)
            d2, hl = divmod(h, 2)
            base = hl * 64
            ps = s_ps.tile([128, 1024], F32, tag="ps", name="ps")
            for c2 in range(2):
                nc.tensor.matmul(
                    ps[:, ts(c2, 512)],
                    kw[d2][base : base + 64, ts(nt, 128)],
                    qw[d2][base : base + 64, mh * 1024 + c2 * 512 : mh * 1024 + (c2 + 1) * 512],
                    start=True, stop=True,
                )
            j = nt % 2
            if j == 0:
                au_cur[0] = au_pool.tile([128, 2048], F16, tag="au", name="au")
            nc.scalar.activation(au_cur[0][:, ts(j, 1024)], ps[:], EXP, scale=0.125)
            if j == 1:
                am = am_pool.tile([128, 2048], F16, tag="am", name="am")
                kt = keep_tiles[(mh, nt // 8)]
                jo = (nt - 1) % 8
                nc.vector.tensor_mul(am[:], au_cur[0][:], kt[:, jo * 1024 : (jo + 2) * 1024])
                am_half[s - 1] = (am, 0)
                am_half[s] = (am, 1)

        def emit_normalize(mh, h, pctx):
            d2, hl = divmod(h, 2)
            base = hl * 64
            csb = csb_pool.tile([64, 1024], F32, tag="csb", name="csb")
            sums = csb_pool.tile([1, 1024], F16, tag="sums", name="sums")
            # parallel drain: DVE and ACT take one half each; sums row via
            # DVE (partition 64 -> 0 shift, as in the baseline kernel)
            nc.vector.tensor_copy(csb[:, 0:512], pctx[0:64, 0:512])
            nc.scalar.copy(csb[:, 512:1024], pctx[0:64, 512:1024])
            nc.vector.tensor_copy(sums[:], pctx[64:65, :])
            rbs = rbs_pool.tile([64, 1024], F32, tag="rbs", name="rbs")
            for c2 in range(2):
                prb = util_ps.tile([128, 512], F32, tag="u", name="prb")
                nc.tensor.matmul(
                    prb[0:64, :], ones64[:], sums[:, ts(c2, 512)],
                    start=True, stop=True,
                )
                nc.vector.reciprocal_approx_fast(rbs[:, ts(c2, 512)], prb[0:64, :])
            nc.vector.tensor_mul(
                ctxs[d2][base : base + 64, mh * 1024 : (mh + 1) * 1024],
                csb[:, :],
                rbs[:],
            )

        def emit_ctx(s):
            mh, h, nt = head_of(s)
            if nt == 0:
                pctx_cur[0] = c_ps.tile([65, 1024], F32, tag="pctx", name="pctx")
            am, half = am_half.pop(s)
            pctx = pctx_cur[0]
            for c2 in range(2):
                nc.tensor.matmul(
                    pctx[:, ts(c2, 512)],
                    vw_sb[:, nt * VSTR + h * 65 : nt * VSTR + (h + 1) * 65],
                    am[:, half * 1024 + c2 * 512 : half * 1024 + (c2 + 1) * 512],
                    start=(nt == 0), stop=(nt == NT - 1),
                )
            if nt == NT - 1:
                emit_normalize(mh, h, pctx)

        for s in range(128 + LAG):
            if s in prefetches:
                dma_keep(*prefetches[s])
            for qta in injections.get(s, ()):
                qta()
            if s < 128:
                emit_scores_exp_mask(s)
            if s >= LAG:
                emit_ctx(s - LAG)

        # tail: out-proj mh1
        for q in range(4):
            oproj_quantum(1, q)()

    nc.finalize()
    return nc


_PROGRAM = None


def _get_program():
    global _PROGRAM
    if _PROGRAM is None:
        _PROGRAM = build_program()
    return _PROGRAM


def _make_in_maps(q, k, v, mask, Wq, Wk, Wv, Wo):
    q = np.asarray(q, dtype=np.float32)
    k = np.asarray(k, dtype=np.float32)
    v = np.asarray(v, dtype=np.float32)
    mask = np.asarray(mask)
    Wq = np.asarray(Wq, dtype=np.float32)
    Wk = np.asarray(Wk, dtype=np.float32)
    Wv = np.asarray(Wv, dtype=np.float32)
    Wo = np.asarray(Wo, dtype=np.float32)

    per_batch = {}
    for b in range(B):
        per_batch[b] = dict(
            qT=np.ascontiguousarray(q[b].T.astype(np.float16)),
            kT=np.ascontiguousarray(k[b].T.astype(np.float16)),
            vT=np.ascontiguousarray(v[b].T.astype(np.float16)),
            keepT=np.ascontiguousarray(
                np.logical_not(mask[b]).T.astype(np.float16)
            ),
        )

    in_maps = []
    for c in range(NCORES):
        b, hg = divmod(c, GROUPS)
        sl = slice(hg * DLOC, (hg + 1) * DLOC)
        in_maps.append(
            dict(
                per_batch[b],
                wqT=np.ascontiguousarray(Wq[sl].T.astype(np.float16)),
                wkT=np.ascontiguousarray(Wk[sl].T.astype(np.float16)),
                wvT=np.ascontiguousarray(Wv[sl].T.astype(np.float16)),
                woT=np.ascontiguousarray(Wo[:, sl].T.astype(np.float16)),
            )
        )
    return in_maps


def _run(in_maps, trace=False):
    nc = _get_program()
    return run_bass_kernel_spmd(
        nc, in_maps, list(range(NCORES)), trace=trace
    )


def _assemble(results):
    out = np.zeros((B, M, E), dtype=np.float32)
    for c in range(NCORES):
        b = c // GROUPS
        out[b] += results[c]["out"].astype(np.float32)
    return out


def kernel(q, k, v, mask, Wq, Wk, Wv, Wo):
    in_maps = _make_in_maps(q, k, v, mask, Wq, Wk, Wv, Wo)
    res = _run(in_maps, trace=False)
    return _assemble(res.results)


def run_profiled(q, k, v, mask, Wq, Wk, Wv, Wo):
    """Like kernel(), but traces execution; returns (out, BassKernelResults)."""
    in_maps = _make_in_maps(q, k, v, mask, Wq, Wk, Wv, Wo)
    res = _run(in_maps, trace=True)
    return _assemble(res.results), res
